# revision 7
# baseline (speedup 1.0000x reference)
import os

if "axon" not in os.environ.get("JAX_PLATFORMS", ""):
    os.environ["JAX_PLATFORMS"] = "axon"

import numpy as np

import concourse.bass as bass
import concourse.bacc as bacc
import concourse.mybir as mybir
import concourse.tile as tile
from concourse.bass_utils import run_bass_kernel_spmd

B, S, HID, NH, LAT = 4, 1024, 2048, 16, 512
HD = 128
NHL = NH // 2
HL = NHL * HD
P = 128
KT_H = HID // P
KT_L = LAT // P
QCW = 512
NQC = S // QCW
SC_SCALE = float(1.0 / np.sqrt(HD))

F32 = mybir.dt.float32
F32R = mybir.dt.float32r

N_CORES = 8


def _r(ap):
    return ap.bitcast(F32R)


def _rope(nc, pool, raw, out_ap, cosT_sb, sinTe_sb):
    sh = pool.tile([P, S], F32, tag="shift")
    nc.sync.dma_start(sh[0:64, :], raw[64:128, :])
    nc.sync.dma_start(sh[64:128, :], raw[0:64, :])
    nc.vector.tensor_mul(out_ap, raw, cosT_sb)
    nc.gpsimd.tensor_mul(sh, sh, sinTe_sb)
    nc.vector.tensor_add(out_ap, out_ap, sh)


def build_bass():
    nc = bacc.Bacc("TRN2", target_bir_lowering=False, debug=False, num_devices=8)

    xT = nc.dram_tensor("xT", [HID, S], F32, kind="ExternalInput")[:]
    wq = nc.dram_tensor("wq", [HID, HL], F32, kind="ExternalInput")[:]
    wdown = nc.dram_tensor("wdown", [HID, LAT], F32, kind="ExternalInput")[:]
    wkup = nc.dram_tensor("wkup", [LAT, HL], F32, kind="ExternalInput")[:]
    wvup = nc.dram_tensor("wvup", [LAT, HL], F32, kind="ExternalInput")[:]
    wo = nc.dram_tensor("wo", [HL, HID], F32, kind="ExternalInput")[:]
    bqd = nc.dram_tensor("bq", [P, NHL], F32, kind="ExternalInput")[:]
    cosTd = nc.dram_tensor("cosT", [P, S], F32, kind="ExternalInput")[:]
    sinTed = nc.dram_tensor("sinTe", [P, S], F32, kind="ExternalInput")[:]
    maskTd = nc.dram_tensor("maskT", [P, 4 * QCW], F32, kind="ExternalInput")[:]
    onescd = nc.dram_tensor("ones_c", [1, P], F32, kind="ExternalInput")[:]
    oneskd = nc.dram_tensor("ones_r", [P, 1], F32, kind="ExternalInput")[:]
    outT = nc.dram_tensor("outT", [HID, S], F32, kind="ExternalOutput")[:]

    with tile.TileContext(nc) as tc:
        with (
            tc.tile_pool(name="consts", bufs=1) as consts,
            tc.tile_pool(name="resident", bufs=1) as resident,
        ):
            cosT_sb = consts.tile([P, S], F32)
            nc.sync.dma_start(cosT_sb, cosTd)
            sinTe_sb = consts.tile([P, S], F32)
            nc.sync.dma_start(sinTe_sb, sinTed)
            mask_sb = consts.tile([P, 4, QCW], F32)
            nc.sync.dma_start(mask_sb, maskTd.rearrange("p (o w) -> p o w", o=4))
            bq_sb = consts.tile([P, NHL], F32)
            nc.sync.dma_start(bq_sb, bqd)
            ones_col = consts.tile([1, P], F32R)
            nc.sync.dma_start(ones_col, onescd.bitcast(F32R))
            ones_k = consts.tile([P, 1], F32R)
            nc.sync.dma_start(ones_k, oneskd.bitcast(F32R))

            latT_sb = resident.tile([P, KT_L, S], F32R)
            qT_sb = resident.tile([P, NHL, S], F32R)

            with (
                tc.tile_pool(name="xp", bufs=1) as xp,
                tc.tile_pool(name="ws1", bufs=3) as ws1,
                tc.tile_pool(name="rope1", bufs=2) as rp1,
                tc.tile_pool(name="pacc1", bufs=6, space="PSUM") as pp1,
            ):
                xT_sb = xp.tile([P, KT_H, S], F32R)
                nc.sync.dma_start(
                    xT_sb, xT.rearrange("(kt p) s -> p kt s", p=P).bitcast(F32R)
                )

                for og in range(2):
                    ps = {}
                    for oi in range(2):
                        for ntc in range(NQC):
                            ps[(oi, ntc)] = pp1.tile([P, QCW], F32, tag="acc", name="acc")
                    for kt in range(KT_H):
                        wt = ws1.tile([P, 2 * P], F32R, tag="wd")
                        nc.sync.dma_start(
                            wt,
                            wdown[kt * P:(kt + 1) * P,
                                  og * 2 * P:(og + 1) * 2 * P].bitcast(F32R),
                        )
                        for oi in range(2):
                            for ntc in range(NQC):
                                nc.tensor.matmul(
                                    ps[(oi, ntc)],
                                    lhsT=wt[:, oi * P:(oi + 1) * P],
                                    rhs=xT_sb[:, kt, ntc * QCW:(ntc + 1) * QCW],
                                    start=(kt == 0),
                                    stop=(kt == KT_H - 1),
                                )
                    for oi in range(2):
                        for ntc in range(NQC):
                            nc.any.tensor_copy(
                                latT_sb[:, og * 2 + oi, ntc * QCW:(ntc + 1) * QCW],
                                ps[(oi, ntc)],
                            )

                for og in range(4):
                    ps = {}
                    for oi in range(2):
                        for ntc in range(NQC):
                            ps[(oi, ntc)] = pp1.tile([P, QCW], F32, tag="acc", name="acc")
                    for kt in range(KT_H):
                        wt = ws1.tile([P, 2 * P], F32R, tag="wq")
                        nc.sync.dma_start(
                            wt,
                            wq[kt * P:(kt + 1) * P,
                               og * 2 * P:(og + 1) * 2 * P].bitcast(F32R),
                        )
                        for oi in range(2):
                            for ntc in range(NQC):
                                nc.tensor.matmul(
                                    ps[(oi, ntc)],
                                    lhsT=wt[:, oi * P:(oi + 1) * P],
                                    rhs=xT_sb[:, kt, ntc * QCW:(ntc + 1) * QCW],
                                    start=(kt == 0),
                                    stop=(kt == KT_H - 1),
                                )
                    for oi in range(2):
                        h = og * 2 + oi
                        raw = rp1.tile([P, S], F32, tag="raw")
                        for ntc in range(NQC):
                            nc.scalar.add(
                                raw[:, ntc * QCW:(ntc + 1) * QCW],
                                ps[(oi, ntc)],
                                bq_sb[:, h:h + 1],
                            )
                        _rope(nc, rp1, raw, qT_sb[:, h, :], cosT_sb, sinTe_sb)

            with tc.tile_pool(name="res2", bufs=1) as res2:
                kT_sb = res2.tile([P, NHL, S], F32R)
                v_sb = res2.tile([P, NHL, HL], F32R)

                with (
                    tc.tile_pool(name="ws2", bufs=3) as ws2,
                    tc.tile_pool(name="rope2", bufs=2) as rp2,
                    tc.tile_pool(name="pacc2", bufs=6, space="PSUM") as pp2,
                ):
                    for og in range(4):
                        ps = {}
                        for oi in range(2):
                            for ntc in range(NQC):
                                ps[(oi, ntc)] = pp2.tile([P, QCW], F32, tag="acc", name="acc")
                        for kt in range(KT_L):
                            wt = ws2.tile([P, 2 * P], F32R, tag="wk")
                            nc.sync.dma_start(
                                wt,
                                wkup[kt * P:(kt + 1) * P,
                                     og * 2 * P:(og + 1) * 2 * P].bitcast(F32R),
                            )
                            for oi in range(2):
                                for ntc in range(NQC):
                                    nc.tensor.matmul(
                                        ps[(oi, ntc)],
                                        lhsT=wt[:, oi * P:(oi + 1) * P],
                                        rhs=latT_sb[:, kt, ntc * QCW:(ntc + 1) * QCW],
                                        start=(kt == 0),
                                        stop=(kt == KT_L - 1),
                                    )
                        for oi in range(2):
                            h = og * 2 + oi
                            raw = rp2.tile([P, S], F32, tag="raw")
                            for ntc in range(NQC):
                                nc.any.tensor_copy(
                                    raw[:, ntc * QCW:(ntc + 1) * QCW], ps[(oi, ntc)]
                                )
                            _rope(nc, rp2, raw, kT_sb[:, h, :], cosT_sb, sinTe_sb)

                    for hlc in range(2):
                        for sg in range(2):
                            ps = {}
                            for si in range(4):
                                ps[si] = pp2.tile([P, QCW], F32, tag="acc", name="acc")
                            for kt in range(KT_L):
                                wt = ws2.tile([P, QCW], F32R, tag="wv")
                                nc.sync.dma_start(
                                    wt,
                                    wvup[kt * P:(kt + 1) * P,
                                         hlc * QCW:(hlc + 1) * QCW].bitcast(F32R),
                                )
                                for si in range(4):
                                    st = sg * 4 + si
                                    nc.tensor.matmul(
                                        ps[si],
                                        lhsT=latT_sb[:, kt, st * P:(st + 1) * P],
                                        rhs=wt,
                                        start=(kt == 0),
                                        stop=(kt == KT_L - 1),
                                    )
                            for si in range(4):
                                st = sg * 4 + si
                                nc.any.tensor_copy(
                                    v_sb[:, st, hlc * QCW:(hlc + 1) * QCW], ps[si]
                                )

                with (
                    tc.tile_pool(name="ctxp", bufs=1) as ctxp,
                    tc.tile_pool(name="exl", bufs=4) as exl,
                    tc.tile_pool(name="small", bufs=2) as small,
                    tc.tile_pool(name="psc", bufs=2, space="PSUM") as psc,
                    tc.tile_pool(name="pctx", bufs=2, space="PSUM") as pctx,
                    tc.tile_pool(name="psum1", bufs=1, space="PSUM") as psum1,
                    tc.tile_pool(name="pbc", bufs=1, space="PSUM") as pbc,
                    tc.tile_pool(name="pout", bufs=2, space="PSUM") as pout,
                    tc.tile_pool(name="wos", bufs=3) as wos,
                    tc.tile_pool(name="outsb", bufs=3) as outsb,
                ):
                    ctxT_sb = ctxp.tile([P, NHL, S], F32R)

                    for qc in range(NQC):
                        for h in range(NHL):
                            nkt = 4 * qc + 4
                            ctx = pctx.tile([P, QCW], F32, tag="ctx")
                            sums = psum1.tile([1, QCW], F32, tag="sums")
                            for kt in range(nkt):
                                sc = psc.tile([P, QCW], F32, tag="sc")
                                nc.tensor.matmul(
                                    sc,
                                    lhsT=kT_sb[:, h, kt * P:(kt + 1) * P],
                                    rhs=qT_sb[:, h, qc * QCW:(qc + 1) * QCW],
                                    start=True,
                                    stop=True,
                                )
                                ex = exl.tile([P, QCW], F32R, tag="ex")
                                nc.scalar.activation(
                                    out=ex, in_=sc,
                                    func=mybir.ActivationFunctionType.Exp,
                                    scale=SC_SCALE,
                                )
                                off = kt - 4 * qc
                                if off >= 0:
                                    nc.gpsimd.tensor_mul(ex, ex, mask_sb[:, off, :])
                                nc.tensor.matmul(
                                    ctx,
                                    lhsT=v_sb[:, kt, h * P:(h + 1) * P],
                                    rhs=ex,
                                    start=(kt == 0),
                                    stop=(kt == nkt - 1),
                                )
                                nc.tensor.matmul(
                                    sums,
                                    lhsT=ones_k,
                                    rhs=ex,
                                    start=(kt == 0),
                                    stop=(kt == nkt - 1),
                                )
                            srow = small.tile([1, QCW], F32R, tag="srow")
                            nc.any.tensor_copy(srow, sums)
                            bc = pbc.tile([P, QCW], F32, tag="bc")
                            nc.tensor.matmul(
                                bc, lhsT=ones_col, rhs=srow,
                                start=True, stop=True,
                            )
                            rec = small.tile([P, QCW], F32, tag="rec")
                            nc.vector.reciprocal(out=rec, in_=bc)
                            nc.vector.tensor_mul(
                                ctxT_sb[:, h, qc * QCW:(qc + 1) * QCW], ctx, rec
                            )

                    for ot in range(HID // P):
                        wt = wos.tile([P, NHL, P], F32R, tag="wo")
                        nc.sync.dma_start(
                            wt,
                            wo[:, ot * P:(ot + 1) * P].rearrange(
                                "(kt p) o -> p kt o", p=P
                            ).bitcast(F32R),
                        )
                        for qc in range(NQC):
                            po = pout.tile([P, QCW], F32, tag="po")
                            for kt in range(NHL):
                                nc.tensor.matmul(
                                    po,
                                    lhsT=wt[:, kt, :],
                                    rhs=ctxT_sb[:, kt, qc * QCW:(qc + 1) * QCW],
                                    start=(kt == 0),
                                    stop=(kt == NHL - 1),
                                )
                            osb = outsb.tile([P, QCW], F32, tag="osb")
                            nc.any.tensor_copy(osb, po)
                            nc.sync.dma_start(
                                outT[ot * P:(ot + 1) * P, qc * QCW:(qc + 1) * QCW],
                                osb,
                            )
    nc.compile()
    return nc



def _host_consts():
    inv_freq = 1.0 / (10000.0 ** (np.arange(0, HD, 2, dtype=np.float64) / HD))
    t = np.arange(S, dtype=np.float64)
    freqs = t[:, None] * inv_freq[None, :]
    emb = np.concatenate([freqs, freqs], axis=-1)
    cosT = np.cos(emb).T.astype(np.float32).copy()
    sinT = np.sin(emb).T.astype(np.float32)
    sinTe = sinT.copy()
    sinTe[:64] *= -1.0
    sinTe = np.ascontiguousarray(sinTe.astype(np.float32))

    ii = np.arange(P)[:, None]
    jj = np.arange(QCW)[None, :]
    masks = np.empty((P, 4, QCW), dtype=np.float32)
    for o in range(4):
        masks[:, o, :] = (jj - 128 * o - ii >= 0).astype(np.float32)
    maskT = np.ascontiguousarray(masks.reshape(P, 4 * QCW))
    return cosT, sinTe, maskT


_CACHE = {}


def _get_built():
    if "nc" not in _CACHE:
        _CACHE["nc"] = build_bass()
        _CACHE["consts"] = _host_consts()
    return _CACHE["nc"], _CACHE["consts"]


def make_in_maps(x, Wq, bq, Wdown, Wk_up, Wv_up, Wo):
    cosT, sinTe, maskT = _get_built()[1]
    in_maps = []
    for c in range(N_CORES):
        b, hg = c // 2, c % 2
        sl = slice(hg * HL, (hg + 1) * HL)
        in_maps.append({
            "xT": np.ascontiguousarray(x[b].T),
            "wq": np.ascontiguousarray(Wq[:, sl]),
            "wdown": np.ascontiguousarray(Wdown),
            "wkup": np.ascontiguousarray(Wk_up[:, sl]),
            "wvup": np.ascontiguousarray(Wv_up[:, sl]),
            "wo": np.ascontiguousarray(Wo[sl, :]),
            "bq": np.ascontiguousarray(bq[sl].reshape(NHL, P).T),
            "cosT": cosT,
            "sinTe": sinTe,
            "maskT": maskT,
            "ones_c": np.ones((1, P), np.float32),
            "ones_r": np.ones((P, 1), np.float32),
        })
    return in_maps


def gather_out(results, bo):
    out = np.empty((B, S, HID), dtype=np.float32)
    for b in range(B):
        acc = results[2 * b]["outT"] + results[2 * b + 1]["outT"]
        out[b] = acc.T + bo[None, :]
    return out


def kernel(x, Wq, bq, Wdown, Wk_up, Wv_up, Wo, bo):
    x = np.asarray(x, dtype=np.float32)
    Wq = np.asarray(Wq, dtype=np.float32)
    bq = np.asarray(bq, dtype=np.float32)
    Wdown = np.asarray(Wdown, dtype=np.float32)
    Wk_up = np.asarray(Wk_up, dtype=np.float32)
    Wv_up = np.asarray(Wv_up, dtype=np.float32)
    Wo = np.asarray(Wo, dtype=np.float32)
    bo = np.asarray(bo, dtype=np.float32)

    nc, _ = _get_built()
    in_maps = make_in_maps(x, Wq, bq, Wdown, Wk_up, Wv_up, Wo)
    res = run_bass_kernel_spmd(nc, in_maps, core_ids=list(range(N_CORES)))
    return gather_out(res.results, bo)


# revision 8
# speedup vs baseline: 290.1864x; 290.1864x over previous
import os

if "axon" not in os.environ.get("JAX_PLATFORMS", ""):
    os.environ["JAX_PLATFORMS"] = "axon"

import numpy as np

import concourse.bass as bass
import concourse.bacc as bacc
import concourse.mybir as mybir
import concourse.tile as tile
from concourse.bass_utils import run_bass_kernel_spmd

B, S, HID, NH, LAT = 4, 1024, 2048, 16, 512
HD = 128
NHL = NH // 2
HL = NHL * HD
P = 128
KT_H = HID // P
KT_L = LAT // P
QCW = 512
NQC = S // QCW
SC_SCALE = float(1.0 / np.sqrt(HD))

F32 = mybir.dt.float32
F32R = mybir.dt.float32r

N_CORES = 8


def _r(ap):
    return ap.bitcast(F32R)


def _rope(nc, pool, raw, out_ap, cosT_sb, sinTe_sb):
    sh = pool.tile([P, S], F32, tag="shift")
    nc.sync.dma_start(sh[0:64, :], raw[64:128, :])
    nc.sync.dma_start(sh[64:128, :], raw[0:64, :])
    nc.vector.tensor_mul(out_ap, raw, cosT_sb)
    nc.gpsimd.tensor_mul(sh, sh, sinTe_sb)
    nc.vector.tensor_add(out_ap, out_ap, sh)


def build_bass():
    nc = bacc.Bacc("TRN2", target_bir_lowering=False, debug=False, num_devices=8)

    xT = nc.dram_tensor("xT", [HID, S], F32, kind="ExternalInput")[:]
    wq = nc.dram_tensor("wq", [HID, HL], F32, kind="ExternalInput")[:]
    wdown = nc.dram_tensor("wdown", [HID, LAT], F32, kind="ExternalInput")[:]
    wkup = nc.dram_tensor("wkup", [LAT, HL], F32, kind="ExternalInput")[:]
    wvup = nc.dram_tensor("wvup", [LAT, HL], F32, kind="ExternalInput")[:]
    wo = nc.dram_tensor("wo", [HL, HID], F32, kind="ExternalInput")[:]
    bqd = nc.dram_tensor("bq", [P, NHL], F32, kind="ExternalInput")[:]
    cosTd = nc.dram_tensor("cosT", [P, S], F32, kind="ExternalInput")[:]
    sinTed = nc.dram_tensor("sinTe", [P, S], F32, kind="ExternalInput")[:]
    maskTd = nc.dram_tensor("maskT", [P, 4 * QCW], F32, kind="ExternalInput")[:]
    onescd = nc.dram_tensor("ones_c", [1, P], F32, kind="ExternalInput")[:]
    oneskd = nc.dram_tensor("ones_r", [P, 1], F32, kind="ExternalInput")[:]
    outT = nc.dram_tensor("outT", [HID, S], F32, kind="ExternalOutput")[:]

    with tile.TileContext(nc) as tc:
        with (
            tc.tile_pool(name="consts", bufs=1) as consts,
            tc.tile_pool(name="resident", bufs=1) as resident,
        ):
            cosT_sb = consts.tile([P, S], F32)
            nc.sync.dma_start(cosT_sb, cosTd)
            sinTe_sb = consts.tile([P, S], F32)
            nc.sync.dma_start(sinTe_sb, sinTed)
            mask_sb = consts.tile([P, 4, QCW], F32)
            nc.sync.dma_start(mask_sb, maskTd.rearrange("p (o w) -> p o w", o=4))
            bq_sb = consts.tile([P, NHL], F32)
            nc.sync.dma_start(bq_sb, bqd)
            ones_col = consts.tile([1, P], F32R)
            nc.sync.dma_start(ones_col, onescd.bitcast(F32R))
            ones_k = consts.tile([P, 1], F32R)
            nc.sync.dma_start(ones_k, oneskd.bitcast(F32R))

            latT_sb = resident.tile([P, KT_L, S], F32R)
            qT_sb = resident.tile([P, NHL, S], F32R)
            kT_sb = resident.tile([P, NHL, S], F32R)

            pacc_cm = tc.tile_pool(name="pacc", bufs=6, space="PSUM")
            pacc = pacc_cm.__enter__()

            with (
                tc.tile_pool(name="xp", bufs=1) as xp,
                tc.tile_pool(name="ws1", bufs=4) as ws1,
                tc.tile_pool(name="rope1", bufs=2) as rp1,
            ):
                xT_sb = xp.tile([P, KT_H, S], F32R)
                for kt in range(KT_H):
                    nc.sync.dma_start(
                        xT_sb[:, kt, :],
                        xT[kt * P:(kt + 1) * P, :].bitcast(F32R),
                    )

                def proj_og(w_dram, rhs_sb, n_kt, og, wtag):
                    ps = {}
                    for oi in range(2):
                        for ntc in range(NQC):
                            ps[(oi, ntc)] = pacc.tile(
                                [P, QCW], F32, tag="acc", name="acc"
                            )
                    for kt in range(n_kt):
                        wt = ws1.tile([P, 2 * P], F32R, tag=wtag, name="wt")
                        nc.sync.dma_start(
                            wt,
                            w_dram[kt * P:(kt + 1) * P,
                                   og * 2 * P:(og + 1) * 2 * P].bitcast(F32R),
                        )
                        for oi in range(2):
                            for ntc in range(NQC):
                                nc.tensor.matmul(
                                    ps[(oi, ntc)],
                                    lhsT=wt[:, oi * P:(oi + 1) * P],
                                    rhs=rhs_sb[:, kt, ntc * QCW:(ntc + 1) * QCW],
                                    start=(kt == 0),
                                    stop=(kt == n_kt - 1),
                                )
                    return ps

                for og in range(2):
                    ps = proj_og(wdown, xT_sb, KT_H, og, "wd")
                    for oi in range(2):
                        for ntc in range(NQC):
                            nc.any.tensor_copy(
                                latT_sb[:, og * 2 + oi, ntc * QCW:(ntc + 1) * QCW],
                                ps[(oi, ntc)],
                            )

                for og in range(4):
                    ps = proj_og(wq, xT_sb, KT_H, og, "wq")
                    for oi in range(2):
                        h = og * 2 + oi
                        raw = rp1.tile([P, S], F32, tag="raw", name="raw")
                        for ntc in range(NQC):
                            nc.scalar.add(
                                raw[:, ntc * QCW:(ntc + 1) * QCW],
                                ps[(oi, ntc)],
                                bq_sb[:, h:h + 1],
                            )
                        _rope(nc, rp1, raw, qT_sb[:, h, :], cosT_sb, sinTe_sb)

                for og in range(4):
                    ps = proj_og(wkup, latT_sb, KT_L, og, "wk")
                    for oi in range(2):
                        h = og * 2 + oi
                        raw = rp1.tile([P, S], F32, tag="raw", name="raw")
                        for ntc in range(NQC):
                            nc.any.tensor_copy(
                                raw[:, ntc * QCW:(ntc + 1) * QCW], ps[(oi, ntc)]
                            )
                        _rope(nc, rp1, raw, kT_sb[:, h, :], cosT_sb, sinTe_sb)

            vpool_cm = tc.tile_pool(name="vres", bufs=1)
            vpool = vpool_cm.__enter__()
            v_sb = vpool.tile([P, NHL, HL], F32R)
            with tc.tile_pool(name="ws2", bufs=4) as ws2:
                for hlc in range(2):
                    for sg in range(2):
                        ps = {}
                        for si in range(4):
                            ps[si] = pacc.tile([P, QCW], F32, tag="acc", name="acc")
                        for kt in range(KT_L):
                            wt = ws2.tile([P, QCW], F32R, tag="wv", name="wt")
                            nc.sync.dma_start(
                                wt,
                                wvup[kt * P:(kt + 1) * P,
                                     hlc * QCW:(hlc + 1) * QCW].bitcast(F32R),
                            )
                            for si in range(4):
                                st = sg * 4 + si
                                nc.tensor.matmul(
                                    ps[si],
                                    lhsT=latT_sb[:, kt, st * P:(st + 1) * P],
                                    rhs=wt,
                                    start=(kt == 0),
                                    stop=(kt == KT_L - 1),
                                )
                        for si in range(4):
                            st = sg * 4 + si
                            nc.any.tensor_copy(
                                v_sb[:, st, hlc * QCW:(hlc + 1) * QCW], ps[si]
                            )

            pacc_cm.__exit__(None, None, None)

            with (
                tc.tile_pool(name="ctxp", bufs=1) as ctxp,
                tc.tile_pool(name="exl", bufs=4) as exl,
                tc.tile_pool(name="small", bufs=2) as small,
                tc.tile_pool(name="psc", bufs=2, space="PSUM") as psc,
                tc.tile_pool(name="pctx", bufs=2, space="PSUM") as pctx,
                tc.tile_pool(name="psum1", bufs=1, space="PSUM") as psum1,
                tc.tile_pool(name="pbc", bufs=1, space="PSUM") as pbc,
                tc.tile_pool(name="pout", bufs=2, space="PSUM") as pout,
                tc.tile_pool(name="wos", bufs=3) as wos,
                tc.tile_pool(name="outsb", bufs=2) as outsb,
            ):
                ctxT_sb = ctxp.tile([P, NHL, S], F32R)

                for qc in range(NQC):
                    for h in range(NHL):
                        nkt = 4 * qc + 4
                        ctx = pctx.tile([P, QCW], F32, tag="ctx")
                        sums = psum1.tile([1, QCW], F32, tag="sums")
                        for kt in range(nkt):
                            sc = psc.tile([P, QCW], F32, tag="sc")
                            nc.tensor.matmul(
                                sc,
                                lhsT=kT_sb[:, h, kt * P:(kt + 1) * P],
                                rhs=qT_sb[:, h, qc * QCW:(qc + 1) * QCW],
                                start=True,
                                stop=True,
                            )
                            ex = exl.tile([P, QCW], F32R, tag="ex")
                            nc.scalar.activation(
                                out=ex, in_=sc,
                                func=mybir.ActivationFunctionType.Exp,
                                scale=SC_SCALE,
                            )
                            off = kt - 4 * qc
                            if off >= 0:
                                nc.gpsimd.tensor_mul(ex, ex, mask_sb[:, off, :])
                            nc.tensor.matmul(
                                ctx,
                                lhsT=v_sb[:, kt, h * P:(h + 1) * P],
                                rhs=ex,
                                start=(kt == 0),
                                stop=(kt == nkt - 1),
                            )
                            nc.tensor.matmul(
                                sums,
                                lhsT=ones_k,
                                rhs=ex,
                                start=(kt == 0),
                                stop=(kt == nkt - 1),
                            )
                        srow = small.tile([1, QCW], F32R, tag="srow")
                        nc.any.tensor_copy(srow, sums)
                        bc = pbc.tile([P, QCW], F32, tag="bc")
                        nc.tensor.matmul(
                            bc, lhsT=ones_col, rhs=srow, start=True, stop=True
                        )
                        rec = small.tile([P, QCW], F32, tag="rec")
                        nc.vector.reciprocal(out=rec, in_=bc)
                        nc.vector.tensor_mul(
                            ctxT_sb[:, h, qc * QCW:(qc + 1) * QCW], ctx, rec
                        )

                for ot in range(HID // P):
                    wt = wos.tile([P, NHL, P], F32R, tag="wo", name="wt")
                    nc.sync.dma_start(
                        wt,
                        wo[:, ot * P:(ot + 1) * P].rearrange(
                            "(kt p) o -> p kt o", p=P
                        ).bitcast(F32R),
                    )
                    for qc in range(NQC):
                        po = pout.tile([P, QCW], F32, tag="po")
                        for kt in range(NHL):
                            nc.tensor.matmul(
                                po,
                                lhsT=wt[:, kt, :],
                                rhs=ctxT_sb[:, kt, qc * QCW:(qc + 1) * QCW],
                                start=(kt == 0),
                                stop=(kt == NHL - 1),
                            )
                        osb = outsb.tile([P, QCW], F32, tag="osb")
                        nc.any.tensor_copy(osb, po)
                        nc.sync.dma_start(
                            outT[ot * P:(ot + 1) * P, qc * QCW:(qc + 1) * QCW],
                            osb,
                        )
            vpool_cm.__exit__(None, None, None)
    nc.compile()
    return nc



def _host_consts():
    inv_freq = 1.0 / (10000.0 ** (np.arange(0, HD, 2, dtype=np.float64) / HD))
    t = np.arange(S, dtype=np.float64)
    freqs = t[:, None] * inv_freq[None, :]
    emb = np.concatenate([freqs, freqs], axis=-1)
    cosT = np.cos(emb).T.astype(np.float32).copy()
    sinT = np.sin(emb).T.astype(np.float32)
    sinTe = sinT.copy()
    sinTe[:64] *= -1.0
    sinTe = np.ascontiguousarray(sinTe.astype(np.float32))

    ii = np.arange(P)[:, None]
    jj = np.arange(QCW)[None, :]
    masks = np.empty((P, 4, QCW), dtype=np.float32)
    for o in range(4):
        masks[:, o, :] = (jj - 128 * o - ii >= 0).astype(np.float32)
    maskT = np.ascontiguousarray(masks.reshape(P, 4 * QCW))
    return cosT, sinTe, maskT


_CACHE = {}


def _get_built():
    if "nc" not in _CACHE:
        _CACHE["nc"] = build_bass()
        _CACHE["consts"] = _host_consts()
    return _CACHE["nc"], _CACHE["consts"]


def make_in_maps(x, Wq, bq, Wdown, Wk_up, Wv_up, Wo):
    cosT, sinTe, maskT = _get_built()[1]
    in_maps = []
    for c in range(N_CORES):
        b, hg = c // 2, c % 2
        sl = slice(hg * HL, (hg + 1) * HL)
        in_maps.append({
            "xT": np.ascontiguousarray(x[b].T),
            "wq": np.ascontiguousarray(Wq[:, sl]),
            "wdown": np.ascontiguousarray(Wdown),
            "wkup": np.ascontiguousarray(Wk_up[:, sl]),
            "wvup": np.ascontiguousarray(Wv_up[:, sl]),
            "wo": np.ascontiguousarray(Wo[sl, :]),
            "bq": np.ascontiguousarray(bq[sl].reshape(NHL, P).T),
            "cosT": cosT,
            "sinTe": sinTe,
            "maskT": maskT,
            "ones_c": np.ones((1, P), np.float32),
            "ones_r": np.ones((P, 1), np.float32),
        })
    return in_maps


def gather_out(results, bo):
    out = np.empty((B, S, HID), dtype=np.float32)
    for b in range(B):
        acc = results[2 * b]["outT"] + results[2 * b + 1]["outT"]
        out[b] = acc.T + bo[None, :]
    return out


def kernel(x, Wq, bq, Wdown, Wk_up, Wv_up, Wo, bo):
    x = np.asarray(x, dtype=np.float32)
    Wq = np.asarray(Wq, dtype=np.float32)
    bq = np.asarray(bq, dtype=np.float32)
    Wdown = np.asarray(Wdown, dtype=np.float32)
    Wk_up = np.asarray(Wk_up, dtype=np.float32)
    Wv_up = np.asarray(Wv_up, dtype=np.float32)
    Wo = np.asarray(Wo, dtype=np.float32)
    bo = np.asarray(bo, dtype=np.float32)

    nc, _ = _get_built()
    in_maps = make_in_maps(x, Wq, bq, Wdown, Wk_up, Wv_up, Wo)
    res = run_bass_kernel_spmd(nc, in_maps, core_ids=list(range(N_CORES)))
    return gather_out(res.results, bo)
